# revision 99
# baseline (speedup 1.0000x reference)
"""Trainium2 Bass kernel for nn_MoE (moe_routing).

Strategy: expert parallelism with sparse token dispatch across 8 NeuronCores
(core e owns expert e), plus an fp8 linearized-gelu decomposition that runs
nearly all FLOPs through DoubleRow fp8 matmuls (4x the bf16 PE rate in the
cost model) while staying inside the 2e-2 relative-error budget.

The accuracy trick: z = x@w1 + b1 has std ~0.018, so gelu is almost linear
there: gelu(z) = 0.5*z + r(z) with r(z) ~ 0.4*z^2, ~70x smaller than z.
Using r' = gelu(z) - 0.5*(x@w1) (the b1 half-term folds into the additive
constant), per expert

    y_e = 0.5*x@(w1_e@w2_e) + r'(z_e)@w2_e + b2_e

The linear term uses a host-precomputed M_e = 0.5*w1_e@w2_e, split into fp8
hi+lo parts (lo quantizes the hi-residual at the SAME scale, recovering
~full precision; the last of the four K-pair quarters of M_lo is dropped,
trading 12% of the error budget for one fewer matmul per output chunk).
mm1's fp8 error only reaches y through r, attenuated by ~0.8*z ~ 0.015, so
one plain-fp8 pass suffices for z.  Measured on the fixed harness input:
absmax error = 78% of the tolerance host-side, 87% on device.

All matmuls are fp8e4m3 DoubleRow (K=256/pass at 0.5 cycles/row): z = x@w1
(32*tb PE cycles/block), x@M_hi + x@M_lo (16+16), r'@w2 (32) -- 96*tb
cycles/block vs 256*tb for dense bf16.  Scales are matched
(s_x*s_M == s_r*s_w2 == 2^28) so xM_hi, xM_lo and r'@w2 accumulate into ONE
PSUM bank per output chunk; the combine is a single (psum + b2*S)*g -> fp16
op.  Gates (softmax prob of the owned expert / S) are computed on the host
-- which already runs the routing -- and shipped as one [1, C] row that a
single partition_broadcast expands, so no gating work sits in front of the
r-cast chain on the Pool engine.

Per block: PE 184 matmuls; ACT 16 gelus (per-hc bias); DVE 8 hc-paired
r-extract stts + 8 combines (Pool cannot read PSUM on HW); Pool 8 hc-paired
scale-casts.  The input stream is ordered by consumption: w1 quartered so
mm1(0) chases the transfers behind warmup/filler matmuls that also complete
the PE p-state ramp, then per-dc-pair (w2, M_hi, M_lo) bundles.  The
emission plan (_plan) weaves half mm1 passes between Y-pass dc-pair chunks
so the ACT gelu chain -- the slowest per-block engine pass -- spreads over
the whole kernel instead of ganging up in front.  Cost-model timeline:
57.9 us vs the 127.5 us bf16 sparse baseline and ~116 us bf16 roofline.
"""

import os
from contextlib import ExitStack

import numpy as np

import concourse.bass as bass
from concourse import bacc
import concourse.mybir as mybir
import concourse.tile as tile
from concourse.bass_utils import run_bass_kernel_spmd

F32 = mybir.dt.float32
F16 = mybir.dt.float16
E4 = mybir.dt.float8e4
AF = mybir.ActivationFunctionType
ALU = mybir.AluOpType
DR = mybir.MatmulPerfMode.DoubleRow

D_MODEL = 1024
D_HEAD = 2048
N_EXPERTS = 8
TOP_K = 2
N_CORES = 8

DC = D_MODEL // 128      # 8  d_model chunks of 128
HC = D_HEAD // 128       # 16 d_head chunks of 128
DCP = DC // 2            # 4  K-pairs over d_model (DoubleRow)
HCP = HC // 2            # 8  K-pairs over d_head

TB = 240                 # max tokens per block (PSUM bank sizing)

# fp8 scaling.  S = S_X*S_M == S_R*S_W2 so the three y-side passes share one
# PSUM accumulation scale.
S_X = 2.0 ** 5
S_W1 = 2.0 ** 17
S_W2 = 2.0 ** 16
S_M = 2.0 ** 23
S_R = 2.0 ** 12
S = S_X * S_M            # 2^28, common PSUM scale of the y pass
S1 = S_X * S_W1          # 2^22, PSUM scale of the z pass

LAST_RESULT = None       # BassKernelResults of the most recent run (for test.py)


def _blocks(C):
    """Token block sizes: 240 throughout with a small (>=64) tail so the
    end-of-kernel drain is short; all 16-aligned.  Falls back to near-equal
    16-aligned blocks."""
    assert C % 16 == 0
    n = (C + TB - 1) // TB
    if n >= 2:
        tail = C - 240 * (n - 1)
        if 64 <= tail <= 240:
            return [240] * (n - 1) + [tail]
    q16 = C // 16
    per = [q16 // n + (1 if i < q16 % n else 0) for i in range(n)]
    return [p * 16 for p in per]


def _cap(max_cnt):
    """Capacity: round max routed count up to 16, minimum one block."""
    return max(64, ((max_cnt + 15) // 16) * 16)


H1 = tuple(range(HCP // 2))
H2 = tuple(range(HCP // 2, HCP))
HA = tuple(range(HCP))
Q1, Q2, Q3, Q4 = (0, 1), (2, 3), (4, 5), (6, 7)


def _plan(nb):
    """Emission plan: list of ("m", b, hcps) mm1 ops and ("y", b, k) Y-pass
    dc-pair chunks.  For the production shape (nb == 5) an ACT-aware cadence
    weaves half mm1 passes between Y chunks so the gelu chain (the slowest
    per-block engine pass) spreads across the whole kernel; other block
    counts use a safe generic order (mm1 up to 4 blocks ahead, whole-block
    Y passes)."""
    P = []

    def m(b, hcps=HA):
        P.append(("m", b, hcps))

    def y(b, k):
        P.append(("y", b, k))

    if nb == 5 and os.environ.get("MOE_SCHED", "v5") == "v5":
        m(0), m(1), m(2, H1)
        y(0, 0), m(2, H2), y(0, 1), y(0, 2), m(3, H1), y(1, 0), y(0, 3), m(3, H2)
        y(1, 1), y(1, 2), y(2, 0), m(4, H1), y(1, 3), y(2, 1), m(4, H2)
        y(2, 2), y(3, 0), y(2, 3), y(3, 1), y(3, 2)
        y(4, 0), y(3, 3), y(4, 1), y(4, 2), y(4, 3)
    else:
        for b in range(min(nb, 4)):
            m(b)
        for b in range(nb):
            if b + 4 < nb:
                m(b + 4)
            for k in range(4):
                y(b, k)
    # every block's mm1 emitted once, every (b, k) chunk emitted once, and
    # each block's mm1 halves all precede its first Y chunk
    seen_m = {}
    seen_y = set()
    for i, (op, b, arg) in enumerate(P):
        if op == "m":
            seen_m[b] = seen_m.get(b, 0) + len(arg)
        else:
            assert seen_m.get(b, 0) == HCP, (b, arg)
            assert (b, arg) not in seen_y
            seen_y.add((b, arg))
    assert set(seen_m) == set(range(nb))
    assert seen_y == {(b, k) for b in range(nb) for k in range(4)}
    return P


def build_nc(C):
    """Build the single-core SPMD Bass program over C routed tokens."""
    blocks = _blocks(C)
    nb = len(blocks)
    nq_tot = (C + 127) // 128
    nc = bacc.Bacc()

    # x, flat block-contiguous fp8: per block [128, DCP, 2, tb]:
    # x_d[p, 8*t0 + (dcp*2+dl)*tb + t] = xq[t0+t, (2*dcp+dl)*128 + p]
    x_d = nc.declare_dram_parameter("x", [128, DC * C], E4, isOutput=False)
    # w1 hc-quartered: [p, q, dcp, dl, j] = w1q[(2*dcp+dl)*128+p, q*512+j]
    w1_d = nc.declare_dram_parameter("w1", [128, 4, DCP, 2, 512], E4, isOutput=False)
    # M hi/lo dc-quartered: [p, q, dcp, dl, j] = Mq[(2*dcp+dl)*128+p, q*256+j]
    mhi_d = nc.declare_dram_parameter("mhi", [128, 4, DCP, 2, 256], E4, isOutput=False)
    # M_lo covers only the first 3 of 4 K-pairs: the dropped quarter's
    # residual costs 12% of the error budget (measured 66% -> 78% on the
    # harness input) and saves one DR matmul per output chunk
    mlo_d = nc.declare_dram_parameter(
        "mlo", [128, 4, DCP - 1, 2, 256], E4, isOutput=False
    )
    # w2 dc-quartered: [p, q, hcp, hl, j] = w2q[(2*hcp+hl)*128+p, q*256+j]
    w2_d = nc.declare_dram_parameter("w2", [128, 4, HCP, 2, 256], E4, isOutput=False)
    # consts: [:, :HC] = b1t (gelu bias), [:, HC:HC+DC] = b2*S per dc chunk
    cb_d = nc.declare_dram_parameter("cb", [128, HC + DC], F32, isOutput=False)
    # host gate row: prob(expert)/S per routed token slot (0 on padding)
    g_d = nc.declare_dram_parameter("g", [1, nq_tot * 128], F32, isOutput=False)
    # out flat block-major fp16: [p, t0*DC + dc*tb + t] = y[t0+t, dc*128+p]
    out_d = nc.declare_dram_parameter("out", [128, DC * C], F16, isOutput=True)

    with tile.TileContext(nc) as tc, ExitStack() as ctx:
        singles = ctx.enter_context(tc.tile_pool(name="singles", bufs=1))
        x_pool = ctx.enter_context(tc.tile_pool(name="xp", bufs=nb))
        g_pool = ctx.enter_context(tc.tile_pool(name="gp", bufs=8))
        r1_pool = ctx.enter_context(tc.tile_pool(name="r1p", bufs=8))
        r_pool = ctx.enter_context(tc.tile_pool(name="rp", bufs=nb))
        y_pool = ctx.enter_context(tc.tile_pool(name="yb", bufs=2 * nb))
        ps_z = ctx.enter_context(tc.tile_pool(name="ps_z", bufs=5, space="PSUM"))
        ps_y = ctx.enter_context(tc.tile_pool(name="ps_y", bufs=3, space="PSUM"))

        cb_sb = singles.tile([128, HC + DC], F32, name="cb")
        g_row = singles.tile([1, nq_tot * 128], F32, name="g_row")
        g_bc = singles.tile([128, nq_tot * 128], F32, name="g_bc")
        w1_sb = [
            singles.tile([128, DCP, 2, 512], E4, name=f"w1q{q}") for q in range(4)
        ]
        mhi_sb = [
            singles.tile([128, DCP, 2, 256], E4, name=f"mhi{q}") for q in range(4)
        ]
        mlo_sb = [
            singles.tile([128, DCP - 1, 2, 256], E4, name=f"mlo{q}")
            for q in range(4)
        ]
        w2_sb = [
            singles.tile([128, HCP, 2, 256], E4, name=f"w2q{q}") for q in range(4)
        ]
        warm_sb = singles.tile([1, TB], E4, name="warm")

        x_sb = {}
        tbs = blocks
        t0s = np.cumsum([0] + blocks).tolist()

        def emit_x_dma(b):
            xt = x_pool.tile([128, DCP, 2, tbs[b]], E4, tag="xt")
            col = DC * t0s[b]
            nc.sync.dma_start(out=xt, in_=x_d[:, col : col + DC * tbs[b]])
            x_sb[b] = xt

        def emit_input_stream():
            # Consumption-ordered FIFO on the sync queue.  w1q0 + x0 lead so
            # mm1(0) starts as early as possible; cb (gelu bias) rides just
            # behind; the remaining w1 quarters interleave with the x blocks
            # (mm1(0) chases them); then the Y-pass (w2, M hi/lo) bundles in
            # per-dc-pair consumption order.
            nc.sync.dma_start(out=w1_sb[0], in_=w1_d[:, 0])
            emit_x_dma(0)
            nc.sync.dma_start(out=cb_sb, in_=cb_d[:])
            nc.sync.dma_start(out=g_row, in_=g_d[:])
            for q in range(1, 4):
                nc.sync.dma_start(out=w1_sb[q], in_=w1_d[:, q])
                if q < nb:
                    emit_x_dma(q)
            # bundle 0 ahead of the remaining x blocks: Y0q0 is gated by the
            # r(0) chain (~12.3us), not by x3/x4
            nc.sync.dma_start(out=w2_sb[0], in_=w2_d[:, 0])
            nc.sync.dma_start(out=mhi_sb[0], in_=mhi_d[:, 0])
            nc.sync.dma_start(out=mlo_sb[0], in_=mlo_d[:, 0])
            for b in range(4, nb):
                emit_x_dma(b)
            for q in range(1, 4):
                nc.sync.dma_start(out=w2_sb[q], in_=w2_d[:, q])
                nc.sync.dma_start(out=mhi_sb[q], in_=mhi_d[:, q])
                nc.sync.dma_start(out=mlo_sb[q], in_=mlo_d[:, q])

        def _x_rhs(b, dcp):
            """[128, 2, tb] moving slice of block b, K-pair dcp."""
            return x_sb[b][:, dcp]

        def emit_pe_warmup(n):
            """Dependency-free fp8 matmuls (~160 ns each at mid p-state) that
            keep PE busy through DMA-gated waits so the p-state ramp reaches
            full speed; also used as gap filler between mm1(0)'s w1-quarter
            chases so the ramp never resets."""
            for _ in range(n):
                pb = ps_y.tile([128, 192], F32, tag="py")
                nc.tensor.matmul(
                    pb,
                    lhsT=warm_sb[:, 0:128],
                    rhs=warm_sb[:, 0:192],
                    start=True,
                    stop=True,
                )

        r_tiles = {}

        def emit_mm1(b, hcps=range(HCP), fills=0):
            """z pass + r' extraction for hc-pairs `hcps`.  Per hc-pair one
            z-PSUM bank holds two [128, tb] groups; gelu is per-hc (per-hc
            bias), the r' stt (DVE) and scale-cast (Pool) run once per pair.
            fills>0 inserts warmup matmuls after each w1 quarter (block 0
            chases the w1 DMAs; fillers keep the p-state ramp alive)."""
            tb = tbs[b]
            if b in r_tiles:
                r_sb = r_tiles[b]
            else:
                r_sb = r_pool.tile([128, HCP, 2, tb], E4, tag="r_sb")
                r_tiles[b] = r_sb
            for hcp in hcps:
                if fills and hcp in (2, 4, 6):
                    emit_pe_warmup(fills)
                ph = ps_z.tile([128, 2, tb], F32, tag="ph")
                g_sb = g_pool.tile([128, 2, tb], F32, tag="g_sb")
                for hl in range(2):
                    hc = 2 * hcp + hl
                    w1t = w1_sb[hc // 4]
                    c0 = (hc % 4) * 128
                    for dcp in range(DCP):
                        nc.tensor.matmul(
                            ph[:, hl],
                            lhsT=w1t[:, dcp, :, c0 : c0 + 128],
                            rhs=_x_rhs(b, dcp),
                            start=(dcp == 0),
                            stop=(dcp == DCP - 1),
                            perf_mode=DR,
                        )
                    # G = gelu(psum/S1 + b1)
                    nc.scalar.activation(
                        g_sb[:, hl],
                        ph[:, hl],
                        AF.Gelu,
                        bias=cb_sb[:, hc : hc + 1],
                        scale=1.0 / S1,
                    )
                # r1 = G - 0.5*psum/S1  (= r + 0.5*b1, absorbed into b2*S)
                r1_sb = r1_pool.tile([128, 2, tb], F32, tag="r1_sb")
                nc.vector.scalar_tensor_tensor(
                    r1_sb, ph, -0.5 / S1, g_sb, op0=ALU.mult, op1=ALU.add
                )
                # r_q = fp8(r1 * S_R)
                nc.gpsimd.tensor_scalar(
                    r_sb[:, hcp], r1_sb, S_R, None, op0=ALU.mult
                )
            return r_sb

        y_cats = {}

        def emit_y_chunk(b, k):
            """Y-pass chunk: the dc pair (2k, 2k+1) of block b.  Per dc one
            PSUM group of 16 DR matmuls -- r'@w2 (8, opens), x@M_hi (4),
            x@M_lo (4, closes) in DMA-bundle order -- then the combine
            (psum + b2*S)*g -> fp16 on DVE.  The fp16 staging is split into
            dc 0-3 / 4-7 half tiles, each DMAed out as soon as its last
            combine lands."""
            r_sb = r_tiles[b]
            tb = tbs[b]
            t0 = t0s[b]
            if k in (0, 2):
                yc = y_pool.tile([128, 4 * tb], F16, tag=f"y_sb{k // 2}")
                y_cats[b, k // 2] = yc
            y_cat = y_cats[b, k // 2]
            for dc in (2 * k, 2 * k + 1):
                py = ps_y.tile([128, tb], F32, tag="py")
                w2t = w2_sb[dc // 2]
                mt_hi = mhi_sb[dc // 2]
                mt_lo = mlo_sb[dc // 2]
                cw = (dc % 2) * 128
                for hcp in range(HCP):
                    nc.tensor.matmul(
                        py,
                        lhsT=w2t[:, hcp, :, cw : cw + 128],
                        rhs=r_sb[:, hcp],
                        start=(hcp == 0),
                        stop=False,
                        perf_mode=DR,
                    )
                for dcp in range(DCP):
                    nc.tensor.matmul(
                        py,
                        lhsT=mt_hi[:, dcp, :, cw : cw + 128],
                        rhs=_x_rhs(b, dcp),
                        start=False,
                        stop=False,
                        perf_mode=DR,
                    )
                for dcp in range(DCP - 1):
                    nc.tensor.matmul(
                        py,
                        lhsT=mt_lo[:, dcp, :, cw : cw + 128],
                        rhs=_x_rhs(b, dcp),
                        start=False,
                        stop=(dcp == DCP - 2),
                        perf_mode=DR,
                    )
                # Pool/GPSIMD cannot read PSUM on HW: combines live on DVE
                d4 = dc % 4
                nc.vector.scalar_tensor_tensor(
                    y_cat[:, d4 * tb : (d4 + 1) * tb],
                    py,
                    cb_sb[:, HC + dc : HC + dc + 1],
                    g_bc[:, t0 : t0 + tb],
                    op0=ALU.add,
                    op1=ALU.mult,
                )
            if k in (1, 3):
                half = k // 2
                y_cat = y_cats.pop((b, half))
                nc.sync.dma_start(
                    out=out_d[
                        :, (t0 * DC + half * 4 * tb) : (t0 * DC + (half + 1) * 4 * tb)
                    ],
                    in_=y_cat,
                )

        warm = int(os.environ.get("MOE_WARM", "18"))
        fills = int(os.environ.get("MOE_FILL", "8"))
        emit_input_stream()
        nc.vector.memset(warm_sb, 1.0)
        nc.gpsimd.partition_broadcast(g_bc, g_row)
        if warm:
            emit_pe_warmup(warm)

        # Software pipeline, paced by the DMA stream and by the r chain
        # (ACT gelus run ~2x slower than the mm1 PE pass, so r(b) lags
        # mm1(b) considerably when mm1 passes are emitted back to back).
        prefill = int(os.environ.get("MOE_PREFILL", "0"))
        for op, b, arg in _plan(nb):
            if op == "m":
                if b in (1, 2) and prefill:
                    emit_pe_warmup(prefill)
                emit_mm1(b, hcps=arg, fills=fills if b == 0 else 0)
            else:
                emit_y_chunk(b, arg)

    return nc


def route_tokens(x2d, gate_w):
    """Host gating: fp32 logits, softmax probs, per-expert routed ids."""
    logits = (x2d @ gate_w.T).astype(np.float32)  # [N, E] fp32
    m = logits.max(axis=1, keepdims=True)
    p = np.exp(logits - m, dtype=np.float32)
    probs = p / p.sum(axis=1, keepdims=True)
    part = np.argpartition(-logits, TOP_K - 1, axis=1)[:, :TOP_K]
    idx_list = []
    for e in range(N_EXPERTS):
        idx_list.append(np.nonzero((part == e).any(axis=1))[0])
    return probs, idx_list


def make_in_maps(x2d, probs, w1, b1, w2, b2, idx_list, C):
    import ml_dtypes

    E8 = ml_dtypes.float8_e4m3
    nq_tot = (C + 127) // 128
    blocks = _blocks(C)

    xq_full = (x2d * S_X).astype(E8)  # one quantization, shared by all cores
    in_maps = []
    for e in range(N_CORES):
        idx = idx_list[e]
        xg = np.zeros((C, D_MODEL), E8)
        xg[: len(idx)] = xq_full[idx]
        # flat block-contiguous: per block [128, DCP, 2, tb]
        parts = []
        t0 = 0
        for tb in blocks:
            blk = xg[t0 : t0 + tb]  # [tb, D]
            parts.append(
                blk.T.reshape(DCP, 2, 128, tb)
                .transpose(2, 0, 1, 3)
                .reshape(128, DC * tb)
            )
            t0 += tb
        xp = np.ascontiguousarray(np.concatenate(parts, axis=1))

        g_full = np.zeros(nq_tot * 128, np.float32)
        g_full[: len(idx)] = probs[idx, e] / S
        g_full = g_full[None, :]

        # w1 [D, H] -> [p, q, dcp, dl, j] = w1q[(2dcp+dl)*128+p, q*512+j]
        w1q = (w1[e] * S_W1).astype(E8)
        w1p = np.ascontiguousarray(
            w1q.reshape(DCP, 2, 128, 4, 512).transpose(2, 3, 0, 1, 4)
        )
        # M = 0.5*w1@w2, scaled, split hi + lo at the same scale
        M = (0.5 * (w1[e].astype(np.float32) @ w2[e].astype(np.float32))) * S_M
        Mhi = M.astype(E8)
        Mlo = (M - Mhi.astype(np.float32)).astype(E8)
        mhip = np.ascontiguousarray(
            Mhi.reshape(DCP, 2, 128, 4, 256).transpose(2, 3, 0, 1, 4)
        )
        mlop = np.ascontiguousarray(
            Mlo.reshape(DCP, 2, 128, 4, 256)[: DCP - 1].transpose(2, 3, 0, 1, 4)
        )
        # w2 [H, D] -> [p, q, hcp, hl, j] = w2q[(2hcp+hl)*128+p, q*256+j]
        w2q = (w2[e] * S_W2).astype(E8)
        w2p = np.ascontiguousarray(
            w2q.reshape(HCP, 2, 128, 4, 256).transpose(2, 3, 0, 1, 4)
        )

        b1t = np.ascontiguousarray(b1[e].reshape(HC, 128).T)  # [128, HC]
        csb = np.ascontiguousarray((b2[e] * S).reshape(DC, 128).T)  # [128, DC]
        cb = np.concatenate([b1t, csb], axis=1).astype(np.float32)

        in_maps.append(
            {
                "x": xp,
                "w1": w1p,
                "mhi": mhip,
                "mlo": mlop,
                "w2": w2p,
                "cb": np.ascontiguousarray(cb),
                "g": g_full,
            }
        )
    return in_maps


def _unpack_out(out, C, D):
    """Device out layout [128, DC*C] fp16 -> [C, D] token-major partial."""
    segs = []
    t0 = 0
    for tb in _blocks(C):
        seg = out[:, t0 * DC : (t0 + tb) * DC].reshape(128, DC, tb)
        segs.append(seg.transpose(2, 1, 0).reshape(tb, D))
        t0 += tb
    return np.concatenate(segs, axis=0)


def kernel(x, gate_w, w1, b1, w2, b2):
    global LAST_RESULT
    x = np.asarray(x, dtype=np.float32)
    B, Sq, D = x.shape
    x2d = np.ascontiguousarray(x.reshape(-1, D))
    gate_w = np.asarray(gate_w, np.float32)

    probs, idx_list = route_tokens(x2d, gate_w)
    C = _cap(max(len(i) for i in idx_list))

    in_maps = make_in_maps(
        x2d,
        probs,
        np.asarray(w1, np.float32),
        np.asarray(b1, np.float32),
        np.asarray(w2, np.float32),
        np.asarray(b2, np.float32),
        idx_list,
        C,
    )
    nc = build_nc(C)
    nc.finalize()
    try:
        res = run_bass_kernel_spmd(nc, in_maps, core_ids=list(range(N_CORES)))
    except ModuleNotFoundError:
        # BASS_TRACE set but the NTFF profile hook isn't importable here:
        # fall back to the untraced PJRT execute path.
        from types import SimpleNamespace

        from concourse import bass2jax

        results = bass2jax.run_bass_via_pjrt(nc, in_maps, n_cores=N_CORES)
        res = SimpleNamespace(
            results=results,
            exec_time_ns=None,
            instructions_and_trace=None,
            profile_json=None,
        )
    LAST_RESULT = res
    y = np.zeros((B * Sq, D), np.float64)
    for e in range(N_CORES):
        idx = idx_list[e]
        part = _unpack_out(res.results[e]["out"], C, D)
        y[idx] += part[: len(idx)].astype(np.float64)
    return y.astype(np.float32).reshape(B, Sq, D)


def _sim_ns(C=None):
    """Cost-model predicted ns (local, no HW)."""
    from concourse.timeline_sim import TimelineSim

    nc = build_nc(C or _cap(1071))
    nc.finalize()
    return TimelineSim(nc, no_exec=True).simulate()


if __name__ == "__main__":
    print(f"predicted {_sim_ns():.0f} ns")

